# revision 12
# baseline (speedup 1.0000x reference)
"""Trainium2 Bass kernel for the audio/visual contrastive loss.

Strategy: K-parallel sharding of the big matmuls. The embedding matmul
E = [A;V] @ [W_a;W_v] contracts K (visual K=69120 dominates). Each of the
8 cores gets a 1/8 slice of the contraction dim (both the activations'
feature columns and the matching weight rows), computes a partial
E.T (512d x 1024samples) with fp32r matmuls (PE transposes bring X into
k-major layout), the partials are AllReduced (bf16 payload), and every
core computes the small loss tail (norms, Gram, exp/log/mean) redundantly
with the raw Gram overlapping the norm-recip chain.

Per-core HBM traffic is ~36 MB (vs ~160 MB for batch-parallel sharding,
which would replicate the 141 MB W_v on every core).
"""

import sys

sys.path.insert(0, "/opt/trn_rl_repo")

import numpy as np

import concourse.bass as bass
import concourse.mybir as mybir
import concourse.tile as tile
from concourse import bacc, bass_utils
from concourse.bass import ts
from concourse.masks import make_identity

N_CORES = 8
B = 256          # batch
S = 2 * B        # samples per modality after the pair-concat
D = 512          # embedding dim
KV_TOT = 3 * 5 * 48 * 96   # 69120 visual features (lower half)
KV = KV_TOT // N_CORES     # 8640 per core
KA_TOT = 1280
KA = KA_TOT // N_CORES     # 160 per core
F32 = mybir.dt.float32
F32R = mybir.dt.float32r
BF16 = mybir.dt.bfloat16
AF = mybir.ActivationFunctionType

_CACHE = {}


def build():
    nc = bacc.Bacc("TRN2", target_bir_lowering=False, debug=False,
                   num_devices=N_CORES)

    xv_d = nc.dram_tensor("xv", [S, KV], F32R, kind="ExternalInput")
    wv_d = nc.dram_tensor("wv", [KV, D], F32R, kind="ExternalInput")
    xa_d = nc.dram_tensor("xa", [S, KA], F32, kind="ExternalInput")
    wa_d = nc.dram_tensor("wa", [KA, D], F32, kind="ExternalInput")
    loss_d = nc.dram_tensor("loss", [1, 1], F32, kind="ExternalOutput")

    # visual k tiles: 67 x 128 + 1 x 64
    kts = [128] * (KV // 128) + ([KV % 128] if KV % 128 else [])
    NKT = len(kts)

    with tile.TileContext(nc) as tc:
        with tc.tile_pool(name="const", bufs=1) as constp, \
             tc.tile_pool(name="emb", bufs=1) as embp:
            ident = constp.tile([128, 128], F32)
            make_identity(nc, ident[:])
            ident_r = constp.tile([128, 128], F32R)
            nc.vector.tensor_copy(ident_r[:], ident[:])
            ones_f = constp.tile([128, 1], F32)
            nc.vector.memset(ones_f[:], 1.0)
            ones_r = constp.tile([128, 1], F32R)
            nc.vector.tensor_copy(ones_r[:], ones_f[:])
            ones_row_f = constp.tile([1, 128], F32)
            nc.vector.memset(ones_row_f[:], 1.0)
            ones_row_r = constp.tile([1, 128], F32R)
            nc.vector.tensor_copy(ones_row_r[:], ones_row_f[:])
            # preload ACT function tables during the k-loop
            warm = constp.tile([1, 4], F32)
            nc.vector.memset(warm[:], 1.0)
            for fn in (AF.Exp, AF.Sqrt, AF.Ln):
                nc.scalar.activation(warm[:], warm[:], fn)

            # E.T partial, (512 d, 1024 s): audio cols 0:512, visual 512:1024
            # bf16 so the AllReduce moves half the bytes.
            e_sb = [embp.tile([128, 2 * S], BF16, tag=f"e{d}", name=f"e_sb{d}")
                    for d in range(4)]

            xv_r = xv_d.ap().rearrange("(a p) k -> p a k", p=128)
            xa_r = xa_d.ap().rearrange("(a p) k -> p a k", p=128)

            # -- Phase A/B: partial E.T (audio first, then visual k-loop) --
            e_sb = [embp.tile([128, 2 * S], BF16, tag=f"e{d}", name=f"e_sb{d}")
                    for d in range(4)]

            with tc.tile_pool(name="xin", bufs=4) as xinp, \
                 tc.tile_pool(name="win", bufs=4) as winp, \
                 tc.tile_pool(name="wr", bufs=4) as wrp, \
                 tc.tile_pool(name="xt", bufs=4) as xtp, \
                 tc.tile_pool(name="pacc", bufs=1, space="PSUM") as paccp, \
                 tc.tile_pool(name="ptr", bufs=3, space="PSUM") as ptrp, \
                 tc.tile_pool(name="pa", bufs=1, space="PSUM") as pap:
                psum_v = [paccp.tile([128, S], F32, tag=f"pv{d}",
                                     name=f"psum_v{d}")
                          for d in range(4)]

                # ---- audio partial (cheap, fills the DMA warmup bubble) ----
                x_a = xinp.tile([128, 4, KA], F32, tag="xa")
                nc.sync.dma_start(out=x_a[:], in_=xa_r[:])
                wa_sb = winp.tile([128, D], F32, tag="wa0")
                nc.sync.dma_start(out=wa_sb[0:128, :], in_=wa_d.ap()[0:128, :])
                wa1_sb = winp.tile([32, D], F32, tag="wa1")
                nc.sync.dma_start(out=wa1_sb[:], in_=wa_d.ap()[128:KA, :])
                war0 = wrp.tile([128, D], F32R, tag="war0")
                nc.scalar.copy(war0[:], wa_sb[:])
                war1 = wrp.tile([32, D], F32R, tag="war1")
                nc.scalar.copy(war1[:], wa1_sb[:])

                pst0 = ptrp.tile([128, 512], F32, tag="pst", name="pst")
                for j in range(4):
                    nc.tensor.transpose(pst0[0:128, ts(j, 128)],
                                        x_a[:, j, 0:128], ident[:])
                xta0 = xtp.tile([128, S], F32R, tag="xta0")
                nc.vector.tensor_copy(xta0[:], pst0[:])
                pst1 = ptrp.tile([128, 512], F32, tag="pst", name="pst")
                for j in range(4):
                    nc.tensor.transpose(pst1[0:32, ts(j, 128)],
                                        x_a[:, j, 128:KA], ident[:])
                xta1 = xtp.tile([32, S], F32R, tag="xta1")
                nc.vector.tensor_copy(xta1[:], pst1[0:32, :])

                for d in range(4):
                    pa_d = pap.tile([128, S], F32)
                    nc.tensor.matmul(pa_d[:], war0[:, ts(d, 128)], xta0[:],
                                     start=True, stop=False)
                    nc.tensor.matmul(pa_d[:], war1[:, ts(d, 128)], xta1[:],
                                     start=False, stop=True)
                    nc.vector.tensor_copy(e_sb[d][:, 0:S], pa_d[:])

                # ---- visual k-loop ----
                k0 = 0
                for kt, kw in enumerate(kts):
                    x_kt = xinp.tile([128, 4, 128], F32R)
                    nc.sync.dma_start(out=x_kt[:, :, 0:kw],
                                      in_=xv_r[:, :, k0:k0 + kw])
                    w_r = winp.tile([128, D], F32R)
                    nc.sync.dma_start(out=w_r[0:kw, :],
                                      in_=wv_d.ap()[k0:k0 + kw, :])

                    pst = ptrp.tile([128, 512], F32R, tag="pst", name="pst")
                    for j in range(4):
                        nc.tensor.transpose(pst[0:kw, ts(j, 128)],
                                            x_kt[:, j, 0:kw], ident_r[:])
                    xt = xtp.tile([128, S], F32R, tag="xt", name="xt")
                    nc.vector.tensor_copy(xt[0:kw, :], pst[0:kw, :])

                    for d in range(4):
                        nc.tensor.matmul(psum_v[d][:],
                                         w_r[0:kw, ts(d, 128)],
                                         xt[0:kw, :],
                                         start=(kt == 0), stop=(kt == NKT - 1))
                    k0 += kw

                for d in range(4):
                    nc.vector.tensor_copy(e_sb[d][:, S:2 * S], psum_v[d][:])
                # re-warm ACT tables during the AllReduce window
                for fn in (AF.Ln, AF.Exp, AF.Sqrt):
                    nc.scalar.activation(warm[:], warm[:], fn)

            # ---------------- Phase C: AllReduce partials (bf16) ----------
            with tc.tile_pool(name="dram", bufs=1, space="DRAM") as dramp, \
                 tc.tile_pool(name="red", bufs=1) as redp:
                in_b = dramp.tile([4 * 128, 2 * S], BF16)
                out_b = dramp.tile([4 * 128, 2 * S], BF16)
                for d in range(4):
                    nc.sync.dma_start(out=in_b[ts(d, 128), :], in_=e_sb[d][:])
                nc.gpsimd.collective_compute(
                    "AllReduce", mybir.AluOpType.add,
                    replica_groups=[list(range(N_CORES))],
                    ins=[in_b.opt()], outs=[out_b.opt()],
                )
                er = []
                for d in range(4):
                    rd = redp.tile([128, 2 * S], BF16, tag=f"r{d}",
                                   name=f"r{d}")
                    nc.sync.dma_start(out=rd[:], in_=out_b[ts(d, 128), :])
                    er.append(rd)

                # ---------------- Phase D: loss tail ----------------
                with tc.tile_pool(name="tail", bufs=1) as tp, \
                     tc.tile_pool(name="ptail", bufs=2, space="PSUM") as ptp, \
                     tc.tile_pool(name="prow", bufs=1, space="PSUM") as prp:
                    # f32r copies of the reduced E.T for the raw Gram work
                    er_r = [tp.tile([128, 2 * S], F32R, tag=f"err{d}",
                                    name=f"er_r{d}")
                            for d in range(4)]
                    sq = [tp.tile([128, 2 * S], F32R, tag=f"sq{d}",
                                  name=f"sq{d}")
                          for d in range(4)]
                    for d in range(4):
                        nc.vector.tensor_copy(er_r[d][:], er[d][:])
                        nc.vector.tensor_mul(sq[d][:], er[d][:], er[d][:])

                    # raw Gram block a x v (starts while norms chain runs)
                    psm = [ptp.tile([128, 512], F32, tag="psm",
                                    name=f"psm{at}")
                           for at in range(4)]
                    for at in range(4):
                        for d in range(4):
                            nc.tensor.matmul(psm[at][:],
                                             er_r[d][:, ts(at, 128)],
                                             er_r[d][:, S:2 * S],
                                             start=(d == 0), stop=(d == 3))

                    # raw diag products (6 pairs x 256 cols)
                    pairs = [(0, 512), (0, 768), (256, 512), (256, 768),
                             (0, 256), (512, 768)]
                    tprod = [tp.tile([128, 6 * 256], F32R, tag=f"tp{d}",
                                     name=f"tprod{d}")
                             for d in range(4)]
                    for d in range(4):
                        for i, (c1, c2) in enumerate(pairs):
                            nc.vector.tensor_mul(
                                tprod[d][:, ts(i, 256)],
                                er_r[d][:, c1:c1 + 256],
                                er_r[d][:, c2:c2 + 256])
                    traw = prp.tile([1, 6 * 256], F32, name="traw")
                    for g in range(3):
                        for d in range(4):
                            nc.tensor.matmul(traw[:, ts(g, 512)], ones_r[:],
                                             tprod[d][:, ts(g, 512)],
                                             start=(d == 0), stop=(d == 3))

                    # norms chain: sq -> norms2 -> sqrt -> 1/norm
                    norm_row = tp.tile([1, 2 * S], F32)
                    for h in range(2):
                        psh = prp.tile([1, 512], F32, tag="row", name="psh", bufs=2)
                        for d in range(4):
                            nc.tensor.matmul(psh[:], ones_r[:],
                                             sq[d][:, ts(h, 512)],
                                             start=(d == 0), stop=(d == 3))
                        nc.scalar.activation(norm_row[:, ts(h, 512)], psh[:],
                                             AF.Sqrt)
                    rn = tp.tile([1, 2 * S], F32)
                    nc.vector.reciprocal(rn[:], norm_row[:])

                    # rn as columns (4 PE transposes) for the exp scale
                    rn_col = tp.tile([128, 4], F32)
                    for at in range(4):
                        prc = prp.tile([128, 1], F32, tag="row", name="prc",
                                       bufs=2)
                        nc.tensor.transpose(prc[:], rn[0:1, ts(at, 128)],
                                            ident[0:1, 0:1])
                        nc.vector.tensor_copy(rn_col[:, at:at + 1], prc[:])

                    # broadcast visual 1/norm along partitions via K=1 matmul
                    rnv_r = tp.tile([1, 512], F32R)
                    nc.vector.tensor_copy(rnv_r[:], rn[0:1, S:2 * S])
                    rnv_bc = tp.tile([128, 512], F32)
                    psb = prp.tile([128, 512], F32, name="psb")
                    nc.tensor.matmul(psb[:], ones_row_r[:], rnv_r[:],
                                     start=True, stop=True)
                    nc.vector.tensor_copy(rnv_bc[:], psb[:])

                    # denominator: rowsum of exp(M * rn_i * rn_j)
                    denp = tp.tile([128, 4], F32)
                    junk = tp.tile([128, 512], F32, tag="junk")
                    mn = tp.tile([128, 512], F32, tag="mn")
                    for at in range(4):
                        nc.vector.tensor_mul(mn[:], psm[at][:], rnv_bc[:])
                        nc.scalar.activation(junk[:], mn[:], AF.Exp,
                                             scale=rn_col[:, at:at + 1],
                                             accum_out=denp[:, at:at + 1])
                    den2 = tp.tile([128, 2], F32)
                    for j in range(2):
                        nc.vector.tensor_add(den2[:, j:j + 1],
                                             denp[:, j:j + 1],
                                             denp[:, j + 2:j + 3])

                    # numerator: exp of scaled diag terms
                    rnp = tp.tile([1, 6 * 256], F32)
                    for i, (c1, c2) in enumerate(pairs):
                        nc.vector.tensor_mul(rnp[:, ts(i, 256)],
                                             rn[0:1, c1:c1 + 256],
                                             rn[0:1, c2:c2 + 256])
                    that = tp.tile([1, 6 * 256], F32)
                    nc.vector.tensor_mul(that[:], traw[:], rnp[:])
                    exp_t = tp.tile([1, 6 * 256], F32)
                    nc.scalar.activation(exp_t[:], that[:], AF.Exp)
                    num = tp.tile([1, 256], F32)
                    nc.vector.tensor_add(num[:], exp_t[:, 0:256],
                                         exp_t[:, 256:512])
                    for i in range(2, 6):
                        nc.vector.tensor_add(num[:], num[:],
                                             exp_t[:, ts(i, 256)])

                    # denominator columns -> row via PE transpose
                    den_row = tp.tile([1, 256], F32)
                    for j in range(2):
                        pdr = prp.tile([1, 128], F32, tag="row", name="pdr", bufs=2)
                        nc.tensor.transpose(pdr[:], den2[:, j:j + 1], ident[:])
                        nc.vector.tensor_copy(den_row[:, ts(j, 128)], pdr[:])

                    rden = tp.tile([1, 256], F32)
                    nc.vector.reciprocal(rden[:], den_row[:])
                    ratio = tp.tile([1, 256], F32)
                    nc.vector.tensor_mul(ratio[:], num[:], rden[:])
                    logr = tp.tile([1, 256], F32)
                    nc.scalar.activation(logr[:], ratio[:], AF.Ln)
                    lsum = tp.tile([1, 1], F32)
                    nc.vector.reduce_sum(lsum[:], logr[:],
                                         axis=mybir.AxisListType.X)
                    loss_sb = tp.tile([1, 1], F32)
                    nc.scalar.activation(loss_sb[:], lsum[:], AF.Copy,
                                         scale=float(-1.0 / B))
                    nc.sync.dma_start(out=loss_d.ap(), in_=loss_sb[:])

    nc.compile()
    return nc


def _get_nc():
    if "nc" not in _CACHE:
        _CACHE["nc"] = build()
    return _CACHE["nc"]


def _shard_inputs(a_1, v_1, a_2, v_2, W_a, W_v):
    # audio: (2b,1,80,16) -> (512, 1280)
    A = np.concatenate([a_1, a_2], axis=0).reshape(S, KA_TOT)
    # visual: (2b,3,5,96,96), keep lower half rows, flatten in native
    # (c,t,r,w) order; W_v rows permuted to match ((t,c)->(c,t) blocks).
    V = np.concatenate([v_1, v_2], axis=0)
    V = V.reshape(S, 15, 96, 96)[:, :, 48:, :].reshape(S, KV_TOT)
    Wvp = np.ascontiguousarray(
        W_v.reshape(5, 3, 48 * 96, D).transpose(1, 0, 2, 3)
    ).reshape(KV_TOT, D)

    in_maps = []
    for c in range(N_CORES):
        in_maps.append({
            "xv": np.ascontiguousarray(V[:, c * KV:(c + 1) * KV]),
            "wv": np.ascontiguousarray(Wvp[c * KV:(c + 1) * KV, :]),
            "xa": np.ascontiguousarray(A[:, c * KA:(c + 1) * KA]),
            "wa": np.ascontiguousarray(W_a[c * KA:(c + 1) * KA, :]),
        })
    return in_maps


def kernel(a_1, v_1, a_2, v_2, W_a, W_v):
    nc = _get_nc()
    in_maps = _shard_inputs(np.asarray(a_1, np.float32),
                            np.asarray(v_1, np.float32),
                            np.asarray(a_2, np.float32),
                            np.asarray(v_2, np.float32),
                            np.asarray(W_a, np.float32),
                            np.asarray(W_v, np.float32))
    res = bass_utils.run_bass_kernel_spmd(nc, in_maps,
                                          core_ids=list(range(N_CORES)))
    return np.asarray(res.results[0]["loss"], np.float32).reshape(())
